# revision 1
# baseline (speedup 1.0000x reference)
"""DeepSets segment-reduce kernel for 8 Trainium2 NeuronCores.

Math:  y = segment_sum(tanh(x @ W1.T + b1), batch) @ W2.T + b2

Strategy (all 8 cores run the SAME program, SPMD; per-core data differs):
  - Host pads every segment to a multiple of B=16 nodes (zero rows), groups
    128 consecutive segments into a "window" (4 windows/core x 8 cores),
    pads every window to a uniform node count, and pre-transposes x so the
    device sees xT [128(h), Nc] per core - no on-device transposes.
  - Device, per 1024-node tile:
      PE:  phiT_pre = W1T_chunk.T @ xT_tile  (2 o-chunks x 2 q-halves, f32)
      ACT: phiT = tanh(psum + b1_chunk)  PSUM->SBUF, bf16 out (fused bias)
      DVE: 3 rounds of pairwise adds (bf16 2x mode) reduce 16-node blocks
           to 8-node half-block sums L05 [128(o-chunk), 128 cols]
      PE:  zT = L05_c0.T @ W2T_c0 + L05_c1.T @ W2T_c1   (fc2 applied to
           half-block sums - fc2 commutes with segment-sum by linearity)
      DVE: copy zT PSUM -> SBUF (bf16)
      PE:  y_win += S_tile.T @ zT   (S = host-built one-hot mapping
           half-block-cols -> segment-cols; accumulates in PSUM per window)
  - Host: y = concat(core outputs) + b2 - npad[g] * (tanh(b1) @ W2.T)
    (exact linear correction for the zero-pad rows, whose phi is tanh(b1)).
"""

import os
import sys

for _p in ("/opt/trn_rl_repo", "/root/.axon_site/_ro/trn_rl_repo"):
    if os.path.isdir(_p) and _p not in sys.path:
        sys.path.append(_p)

import numpy as np
import ml_dtypes

G = 4096          # segments
H = 128           # input feature dim
O = 256           # hidden dim (2*H)
B = 16            # segment padding granularity (nodes)
HB = 8            # half-block: one L05 column sums HB nodes
T = 1024          # main-loop tile, in nodes
SEGS_PER_WIN = 128
N_CORES = 8
WINS_PER_CORE = 4
N_WINS = N_CORES * WINS_PER_CORE  # 32

_BF16 = ml_dtypes.bfloat16


def _prep_host(x, batch):
    """Pad/shard/transpose inputs. Returns per-core arrays + metadata."""
    x = np.asarray(x, dtype=np.float32)
    batch = np.asarray(batch, dtype=np.int64)
    N = x.shape[0]

    cnt = np.bincount(batch, minlength=G).astype(np.int64)     # [G]
    plen = ((cnt + B - 1) // B) * B                            # [G]

    win_starts = np.arange(0, G, SEGS_PER_WIN)
    win_nodes = np.add.reduceat(plen, win_starts)              # [32]
    Lw = int(((win_nodes.max() + T - 1) // T) * T)             # nodes/window
    Nc = WINS_PER_CORE * Lw                                    # nodes/core

    # exclusive cumsum of plen within each window
    plen_c = np.cumsum(plen)
    seg_pad_start = np.concatenate(([0], plen_c[:-1]))         # global padded start
    win_of_seg = np.arange(G) // SEGS_PER_WIN
    win_pad_base = np.concatenate(([0], plen_c[win_starts[1:] - 1]))
    seg_start_in_win = seg_pad_start - win_pad_base[win_of_seg]  # [G]

    # destination position of each node
    seg_first = np.concatenate(([0], np.cumsum(cnt)[:-1]))     # orig first node
    idx_in_seg = np.arange(N) - seg_first[batch]
    core_of_node = (batch // (SEGS_PER_WIN * WINS_PER_CORE)).astype(np.int64)
    win_in_core = win_of_seg[batch] % WINS_PER_CORE
    pos = win_in_core * Lw + seg_start_in_win[batch] + idx_in_seg  # [N]

    xT = np.zeros((N_CORES, H, Nc), dtype=np.float32)
    # scatter: xT[core, :, pos] = x[n]
    flat = core_of_node * Nc + pos
    xpad = np.zeros((N_CORES * Nc, H), dtype=np.float32)
    xpad[flat] = x
    xT[:] = xpad.reshape(N_CORES, Nc, H).transpose(0, 2, 1)

    # S matrices: per core, per ltile (=128 l05 cols =1024 nodes):
    # S[lrow, segcol] = 1 if l05 col belongs to that segment (col = seg % 128)
    L = Nc // HB                      # l05 cols per core
    ntiles = Nc // T                  # main tiles per core == ltiles per core
    seg_of_col = np.full((N_CORES, L), -1, dtype=np.int64)
    # for each segment: cols [start/HB, (start+plen)/HB) in its core
    core_of_seg = np.arange(G) // (SEGS_PER_WIN * WINS_PER_CORE)
    col_start = (win_of_seg % WINS_PER_CORE) * (Lw // HB) + seg_start_in_win // HB
    ncols_seg = plen // HB
    for g in range(G):
        if ncols_seg[g] > 0:
            c = core_of_seg[g]
            s = col_start[g]
            seg_of_col[c, s:s + ncols_seg[g]] = g % SEGS_PER_WIN
    S = np.zeros((N_CORES, ntiles, SEGS_PER_WIN, SEGS_PER_WIN), dtype=np.float32)
    lt_of_col = (np.arange(L) // SEGS_PER_WIN)
    row_of_col = np.arange(L) % SEGS_PER_WIN
    for c in range(N_CORES):
        mask = seg_of_col[c] >= 0
        S[c, lt_of_col[mask], row_of_col[mask], seg_of_col[c, mask]] = 1.0
    S = S.astype(_BF16)

    npad = (plen - cnt).astype(np.float32)                     # [G]
    return xT, S, Nc, ntiles, npad


def _build_program(Nc, ntiles):
    """Build + compile the (uniform, SPMD) Bass/Tile program for one core."""
    from contextlib import ExitStack
    import concourse.tile as tile
    from concourse import bacc, mybir

    f32 = mybir.dt.float32
    bf16 = mybir.dt.bfloat16
    lt_per_win = ntiles // WINS_PER_CORE

    nc = bacc.Bacc("TRN2", target_bir_lowering=False, debug=False)
    x_d = nc.dram_tensor("xt", [H, Nc], f32, kind="ExternalInput").ap()
    w1t_d = nc.dram_tensor("w1t", [H, O], f32, kind="ExternalInput").ap()
    w2t_d = nc.dram_tensor("w2t", [2, H, H], bf16, kind="ExternalInput").ap()
    b1_d = nc.dram_tensor("b1c", [2, H, 1], f32, kind="ExternalInput").ap()
    s_d = nc.dram_tensor("smat", [ntiles, SEGS_PER_WIN, SEGS_PER_WIN], bf16,
                         kind="ExternalInput").ap()
    y_d = nc.dram_tensor("y", [WINS_PER_CORE * SEGS_PER_WIN, H], f32,
                         kind="ExternalOutput").ap()

    with tile.TileContext(nc) as tc:
        with ExitStack() as ctx:
            singles = ctx.enter_context(tc.tile_pool(name="singles", bufs=1))
            xpool = ctx.enter_context(tc.tile_pool(name="xpool", bufs=4))
            phipool = ctx.enter_context(tc.tile_pool(name="phipool", bufs=2))
            treepool = ctx.enter_context(tc.tile_pool(name="treepool", bufs=2))
            l05pool = ctx.enter_context(tc.tile_pool(name="l05pool", bufs=3))
            spool = ctx.enter_context(tc.tile_pool(name="spool", bufs=3))
            zpool = ctx.enter_context(tc.tile_pool(name="zpool", bufs=3))
            ypool = ctx.enter_context(tc.tile_pool(name="ypool", bufs=2))
            pspool = ctx.enter_context(
                tc.tile_pool(name="pspool", bufs=1, space="PSUM"))
            zps_pool = ctx.enter_context(
                tc.tile_pool(name="zps", bufs=2, space="PSUM"))
            yps_pool = ctx.enter_context(
                tc.tile_pool(name="yps", bufs=2, space="PSUM"))

            w1t = singles.tile([H, O], f32)
            nc.sync.dma_start(out=w1t[:], in_=w1t_d[:])
            w2t0 = singles.tile([H, H], bf16)
            nc.sync.dma_start(out=w2t0[:], in_=w2t_d[0])
            w2t1 = singles.tile([H, H], bf16)
            nc.sync.dma_start(out=w2t1[:], in_=w2t_d[1])
            b1c0 = singles.tile([H, 1], f32)
            nc.sync.dma_start(out=b1c0[:], in_=b1_d[0])
            b1c1 = singles.tile([H, 1], f32)
            nc.sync.dma_start(out=b1c1[:], in_=b1_d[1])

            yps = None
            for t in range(ntiles):
                w = t // lt_per_win
                # ---- load xT tile
                xt = xpool.tile([H, T], f32)
                nc.sync.dma_start(out=xt[:], in_=x_d[:, t * T:(t + 1) * T])

                # ---- fc1 (f32): psA = W1T[:,0:128].T @ xt ; psB = chunk1
                psA = pspool.tile([H, T], f32, tag="psA")
                psB = pspool.tile([H, T], f32, tag="psB")
                for q in range(T // 512):
                    sl = slice(q * 512, (q + 1) * 512)
                    nc.tensor.matmul(psA[:, sl], lhsT=w1t[:, 0:H],
                                     rhs=xt[:, sl], start=True, stop=True)
                for q in range(T // 512):
                    sl = slice(q * 512, (q + 1) * 512)
                    nc.tensor.matmul(psB[:, sl], lhsT=w1t[:, H:O],
                                     rhs=xt[:, sl], start=True, stop=True)

                # ---- tanh + bias (ACT), bf16 out
                phi0 = phipool.tile([H, T], bf16, tag="phi0")
                nc.scalar.activation(phi0[:], psA[:],
                                     mybir.ActivationFunctionType.Tanh,
                                     bias=b1c0[:], scale=1.0)
                phi1 = phipool.tile([H, T], bf16, tag="phi1")
                nc.scalar.activation(phi1[:], psB[:],
                                     mybir.ActivationFunctionType.Tanh,
                                     bias=b1c1[:], scale=1.0)

                # ---- DVE tree: 16 -> 8 -> 4 -> 2 per block (bf16 2x mode)
                NB = T // B  # blocks in tile
                l05s = []
                for ci, phi in enumerate((phi0, phi1)):
                    p3 = phi.rearrange("p (nb w) -> p nb w", w=B)
                    s1 = treepool.tile([H, NB, 8], bf16, tag=f"s1c{ci}")
                    nc.vector.tensor_add(s1[:], p3[:, :, 0:8], p3[:, :, 8:16])
                    s2 = treepool.tile([H, NB, 4], bf16, tag=f"s2c{ci}")
                    nc.vector.tensor_add(s2[:], s1[:, :, 0:4], s1[:, :, 4:8])
                    l05 = l05pool.tile([H, NB, 2], bf16, tag=f"l05c{ci}")
                    nc.vector.tensor_add(l05[:], s2[:, :, 0:2], s2[:, :, 2:4])
                    l05s.append(l05)

                # ---- fc2 on half-block sums -> zT [128(l05col), 128(h)]
                zps = zps_pool.tile([SEGS_PER_WIN, H], f32)
                nc.tensor.matmul(zps[:], lhsT=l05s[0].rearrange("p a b -> p (a b)"),
                                 rhs=w2t0[:], start=True, stop=False)
                nc.tensor.matmul(zps[:], lhsT=l05s[1].rearrange("p a b -> p (a b)"),
                                 rhs=w2t1[:], start=False, stop=True)
                zsb = zpool.tile([SEGS_PER_WIN, H], bf16)
                nc.vector.tensor_copy(zsb[:], zps[:])

                # ---- combine into per-window segment sums (PSUM accumulate)
                st = spool.tile([SEGS_PER_WIN, SEGS_PER_WIN], bf16)
                nc.sync.dma_start(out=st[:], in_=s_d[t])
                if t % lt_per_win == 0:
                    yps = yps_pool.tile([SEGS_PER_WIN, H], f32)
                nc.tensor.matmul(yps[:], lhsT=st[:], rhs=zsb[:],
                                 start=(t % lt_per_win == 0),
                                 stop=(t % lt_per_win == lt_per_win - 1))
                if t % lt_per_win == lt_per_win - 1:
                    ysb = ypool.tile([SEGS_PER_WIN, H], f32)
                    nc.vector.tensor_copy(ysb[:], yps[:])
                    nc.sync.dma_start(
                        out=y_d[w * SEGS_PER_WIN:(w + 1) * SEGS_PER_WIN, :],
                        in_=ysb[:])

    nc.compile()
    return nc


class _Runner:
    """Persistent jitted SPMD executor over jax.devices()[:8]."""

    def __init__(self, nc):
        import jax
        from jax.sharding import Mesh, PartitionSpec
        from jax.experimental.shard_map import shard_map
        from concourse import mybir
        from concourse.bass2jax import (_bass_exec_p, install_neuronx_cc_hook,
                                        partition_id_tensor)
        install_neuronx_cc_hook()
        self.jax = jax
        self.nc = nc
        in_names, out_names, out_avals, zero_outs = [], [], [], []
        partition_name = (nc.partition_id_tensor.name
                          if nc.partition_id_tensor else None)
        for alloc in nc.m.functions[0].allocations:
            if not isinstance(alloc, mybir.MemoryLocationSet):
                continue
            name = alloc.memorylocations[0].name
            if alloc.kind == "ExternalInput":
                if name != partition_name:
                    in_names.append(name)
            elif alloc.kind == "ExternalOutput":
                shape = tuple(alloc.tensor_shape)
                dtype = mybir.dt.np(alloc.dtype)
                out_names.append(name)
                out_avals.append(jax.core.ShapedArray(shape, dtype))
                zero_outs.append(np.zeros(shape, dtype))
        self.in_names, self.out_names = in_names, out_names
        self.out_avals, self.zero_outs = out_avals, zero_outs
        all_in = in_names + out_names + ([partition_name] if partition_name else [])

        def _body(*args):
            operands = list(args)
            if partition_name is not None:
                operands.append(partition_id_tensor())
            return tuple(_bass_exec_p.bind(
                *operands,
                out_avals=tuple(out_avals),
                in_names=tuple(all_in),
                out_names=tuple(out_names),
                lowering_input_output_aliases=(),
                sim_require_finite=True,
                sim_require_nnan=True,
                nc=nc,
            ))

        devices = jax.devices()[:N_CORES]
        self.mesh = Mesh(np.asarray(devices), ("core",))
        n_args = len(in_names) + len(out_names)
        self.fn = jax.jit(
            shard_map(_body, mesh=self.mesh,
                      in_specs=(PartitionSpec("core"),) * n_args,
                      out_specs=(PartitionSpec("core"),) * len(out_names),
                      check_rep=False),
            keep_unused=True,
        )

    def place_inputs(self, in_maps):
        from jax.sharding import NamedSharding, PartitionSpec
        sharding = NamedSharding(self.mesh, PartitionSpec("core"))
        args = []
        for name in self.in_names:
            concat = np.concatenate(
                [np.asarray(m[name]) for m in in_maps], axis=0)
            args.append(self.jax.device_put(concat, sharding))
        for z in self.zero_outs:
            concat = np.zeros((N_CORES * z.shape[0], *z.shape[1:]), z.dtype)
            args.append(self.jax.device_put(concat, sharding))
        return args

    def run(self, args):
        outs = self.fn(*args)
        self.jax.block_until_ready(outs)
        return outs

    def results(self, outs):
        res = []
        for c in range(N_CORES):
            d = {}
            for i, name in enumerate(self.out_names):
                d[name] = np.asarray(outs[i]).reshape(
                    N_CORES, *self.out_avals[i].shape)[c]
            res.append(d)
        return res


_CACHE = {}


def _get_runner(Nc, ntiles):
    key = (Nc, ntiles)
    if key not in _CACHE:
        nc = _build_program(Nc, ntiles)
        _CACHE[key] = _Runner(nc)
    return _CACHE[key]


def _make_in_maps(x, batch, W1, b1, W2):
    xT, S, Nc, ntiles, npad = _prep_host(x, batch)
    W1 = np.asarray(W1, np.float32)
    W2 = np.asarray(W2, np.float32)
    b1 = np.asarray(b1, np.float32)
    w1t = np.ascontiguousarray(W1.T)                     # [128, 256]
    w2t = np.ascontiguousarray(W2.T).reshape(2, H, H).astype(_BF16)
    b1c = b1.reshape(2, H, 1)
    in_maps = []
    for c in range(N_CORES):
        in_maps.append({
            "xt": xT[c], "w1t": w1t, "w2t": w2t, "b1c": b1c, "smat": S[c],
        })
    return in_maps, Nc, ntiles, npad


def kernel(x, batch, W1, b1, W2, b2):
    x = np.asarray(x, np.float32)
    batch_np = np.asarray(batch)
    b1_np = np.asarray(b1, np.float32)
    b2_np = np.asarray(b2, np.float32)
    W2_np = np.asarray(W2, np.float32)

    in_maps, Nc, ntiles, npad = _make_in_maps(x, batch_np, W1, b1_np, W2_np)
    runner = _get_runner(Nc, ntiles)
    args = runner.place_inputs(in_maps)
    outs = runner.run(args)
    res = runner.results(outs)

    y = np.concatenate([res[c]["y"] for c in range(N_CORES)], axis=0)  # [4096,128]
    corr = np.tanh(b1_np.astype(np.float64)) @ W2_np.astype(np.float64).T  # [128]
    y = y + b2_np[None, :] - npad[:, None] * corr[None, :].astype(np.float32)
    return y.astype(np.float32)


# revision 3
# speedup vs baseline: 1.7805x; 1.7805x over previous
"""DeepSets segment-reduce kernel for 8 Trainium2 NeuronCores.

Math:  y = segment_sum(tanh(x @ W1.T + b1), batch) @ W2.T + b2

Strategy (all 8 cores run the SAME program, SPMD; per-core data differs):
  - Host pads every segment to a multiple of B=16 nodes (zero rows), groups
    128 consecutive segments into a "window" (4 windows/core x 8 cores),
    pads every window to a uniform node count, and pre-transposes x so the
    device sees xT [128(h), Nc] per core - no on-device transposes.
  - Device, per 1024-node tile:
      PE:  phiT_pre = W1T_chunk.T @ xT_tile  (2 o-chunks x 2 q-halves, f32)
      ACT: phiT = tanh(psum + b1_chunk)  PSUM->SBUF, bf16 out (fused bias)
      DVE: 3 rounds of pairwise adds (bf16 2x mode) reduce 16-node blocks
           to 8-node half-block sums L05 [128(o-chunk), 128 cols]
      PE:  zT = L05_c0.T @ W2T_c0 + L05_c1.T @ W2T_c1   (fc2 applied to
           half-block sums - fc2 commutes with segment-sum by linearity)
      DVE: copy zT PSUM -> SBUF (bf16)
      PE:  y_win += S_tile.T @ zT   (S = host-built one-hot mapping
           half-block-cols -> segment-cols; accumulates in PSUM per window)
  - Host: y = concat(core outputs) + b2 - npad[g] * (tanh(b1) @ W2.T)
    (exact linear correction for the zero-pad rows, whose phi is tanh(b1)).
"""

import os
import sys

for _p in ("/opt/trn_rl_repo", "/root/.axon_site/_ro/trn_rl_repo"):
    if os.path.isdir(_p) and _p not in sys.path:
        sys.path.append(_p)

import numpy as np
import ml_dtypes

G = 4096          # segments
H = 128           # input feature dim
O = 256           # hidden dim (2*H)
B = 16            # segment padding granularity (nodes)
HB = 8            # half-block: one L05 column sums HB nodes
T = 1024          # main-loop tile, in nodes
SEGS_PER_WIN = 128
N_CORES = 8
WINS_PER_CORE = 4
N_WINS = N_CORES * WINS_PER_CORE  # 32

_BF16 = ml_dtypes.bfloat16


def _prep_host(x, batch):
    """Pad/shard/transpose inputs. Returns per-core arrays + metadata."""
    x = np.asarray(x, dtype=np.float32)
    batch = np.asarray(batch, dtype=np.int64)
    N = x.shape[0]

    cnt = np.bincount(batch, minlength=G).astype(np.int64)     # [G]
    plen = ((cnt + B - 1) // B) * B                            # [G]

    win_starts = np.arange(0, G, SEGS_PER_WIN)
    win_nodes = np.add.reduceat(plen, win_starts)              # [32]
    Lw = int(((win_nodes.max() + T - 1) // T) * T)             # nodes/window
    Nc = WINS_PER_CORE * Lw                                    # nodes/core

    # exclusive cumsum of plen within each window
    plen_c = np.cumsum(plen)
    seg_pad_start = np.concatenate(([0], plen_c[:-1]))         # global padded start
    win_of_seg = np.arange(G) // SEGS_PER_WIN
    win_pad_base = np.concatenate(([0], plen_c[win_starts[1:] - 1]))
    seg_start_in_win = seg_pad_start - win_pad_base[win_of_seg]  # [G]

    # destination position of each node
    seg_first = np.concatenate(([0], np.cumsum(cnt)[:-1]))     # orig first node
    idx_in_seg = np.arange(N) - seg_first[batch]
    core_of_node = (batch // (SEGS_PER_WIN * WINS_PER_CORE)).astype(np.int64)
    win_in_core = win_of_seg[batch] % WINS_PER_CORE
    pos = win_in_core * Lw + seg_start_in_win[batch] + idx_in_seg  # [N]

    # scatter: xT[core, :, pos] = x[n]  (bf16 for full-rate PE + half DMA)
    flat = core_of_node * Nc + pos
    xpad = np.zeros((N_CORES * Nc, H), dtype=_BF16)
    xpad[flat] = x.astype(_BF16)
    xT = np.ascontiguousarray(xpad.reshape(N_CORES, Nc, H).transpose(0, 2, 1))

    # S matrices: per core, per ltile (=128 l05 cols =1024 nodes):
    # S[lrow, segcol] = 1 if l05 col belongs to that segment (col = seg % 128)
    L = Nc // HB                      # l05 cols per core
    ntiles = Nc // T                  # main tiles per core == ltiles per core
    seg_of_col = np.full((N_CORES, L), -1, dtype=np.int64)
    # for each segment: cols [start/HB, (start+plen)/HB) in its core
    core_of_seg = np.arange(G) // (SEGS_PER_WIN * WINS_PER_CORE)
    col_start = (win_of_seg % WINS_PER_CORE) * (Lw // HB) + seg_start_in_win // HB
    ncols_seg = plen // HB
    for g in range(G):
        if ncols_seg[g] > 0:
            c = core_of_seg[g]
            s = col_start[g]
            seg_of_col[c, s:s + ncols_seg[g]] = g % SEGS_PER_WIN
    S = np.zeros((N_CORES, ntiles, SEGS_PER_WIN, SEGS_PER_WIN), dtype=np.float32)
    lt_of_col = (np.arange(L) // SEGS_PER_WIN)
    row_of_col = np.arange(L) % SEGS_PER_WIN
    for c in range(N_CORES):
        mask = seg_of_col[c] >= 0
        S[c, lt_of_col[mask], row_of_col[mask], seg_of_col[c, mask]] = 1.0
    S = S.astype(_BF16)

    npad = (plen - cnt).astype(np.float32)                     # [G]
    return xT, S, Nc, ntiles, npad


def _build_program(Nc, ntiles):
    """Build + compile the (uniform, SPMD) Bass/Tile program for one core."""
    from contextlib import ExitStack
    import concourse.tile as tile
    from concourse import bacc, mybir

    f32 = mybir.dt.float32
    bf16 = mybir.dt.bfloat16
    lt_per_win = ntiles // WINS_PER_CORE

    nc = bacc.Bacc("TRN2", target_bir_lowering=False, debug=False)
    x_d = nc.dram_tensor("xt", [H, Nc], bf16, kind="ExternalInput").ap()
    w1t_d = nc.dram_tensor("w1t", [H, O], bf16, kind="ExternalInput").ap()
    w2t_d = nc.dram_tensor("w2t", [2, H, H], bf16, kind="ExternalInput").ap()
    b1_d = nc.dram_tensor("b1c", [2, H, 1], f32, kind="ExternalInput").ap()
    s_d = nc.dram_tensor("smat", [ntiles, SEGS_PER_WIN, SEGS_PER_WIN], bf16,
                         kind="ExternalInput").ap()
    y_d = nc.dram_tensor("y", [WINS_PER_CORE * SEGS_PER_WIN, H], f32,
                         kind="ExternalOutput").ap()

    with tile.TileContext(nc) as tc:
        with ExitStack() as ctx:
            singles = ctx.enter_context(tc.tile_pool(name="singles", bufs=1))
            xpool = ctx.enter_context(tc.tile_pool(name="xpool", bufs=4))
            phipool = ctx.enter_context(tc.tile_pool(name="phipool", bufs=2))
            treepool = ctx.enter_context(tc.tile_pool(name="treepool", bufs=2))
            l05pool = ctx.enter_context(tc.tile_pool(name="l05pool", bufs=3))
            spool = ctx.enter_context(tc.tile_pool(name="spool", bufs=3))
            zpool = ctx.enter_context(tc.tile_pool(name="zpool", bufs=3))
            ypool = ctx.enter_context(tc.tile_pool(name="ypool", bufs=2))
            pspool = ctx.enter_context(
                tc.tile_pool(name="pspool", bufs=1, space="PSUM"))
            zps_pool = ctx.enter_context(
                tc.tile_pool(name="zps", bufs=2, space="PSUM"))
            yps_pool = ctx.enter_context(
                tc.tile_pool(name="yps", bufs=2, space="PSUM"))

            w1t = singles.tile([H, O], bf16)
            nc.sync.dma_start(out=w1t[:], in_=w1t_d[:])
            w2t0 = singles.tile([H, H], bf16)
            nc.sync.dma_start(out=w2t0[:], in_=w2t_d[0])
            w2t1 = singles.tile([H, H], bf16)
            nc.sync.dma_start(out=w2t1[:], in_=w2t_d[1])
            b1c0 = singles.tile([H, 1], f32)
            nc.sync.dma_start(out=b1c0[:], in_=b1_d[0])
            b1c1 = singles.tile([H, 1], f32)
            nc.sync.dma_start(out=b1c1[:], in_=b1_d[1])

            yps = None
            for t in range(ntiles):
                w = t // lt_per_win
                # ---- load xT tile
                xt = xpool.tile([H, T], bf16)
                nc.sync.dma_start(out=xt[:], in_=x_d[:, t * T:(t + 1) * T])

                # ---- fc1 (f32): psA = W1T[:,0:128].T @ xt ; psB = chunk1
                psA = pspool.tile([H, T], f32, tag="psA")
                psB = pspool.tile([H, T], f32, tag="psB")
                for q in range(T // 512):
                    sl = slice(q * 512, (q + 1) * 512)
                    nc.tensor.matmul(psA[:, sl], lhsT=w1t[:, 0:H],
                                     rhs=xt[:, sl], start=True, stop=True)
                for q in range(T // 512):
                    sl = slice(q * 512, (q + 1) * 512)
                    nc.tensor.matmul(psB[:, sl], lhsT=w1t[:, H:O],
                                     rhs=xt[:, sl], start=True, stop=True)

                # ---- tanh + bias (ACT), bf16 out
                phi0 = phipool.tile([H, T], bf16, tag="phi0")
                nc.scalar.activation(phi0[:], psA[:],
                                     mybir.ActivationFunctionType.Tanh,
                                     bias=b1c0[:], scale=1.0)
                phi1 = phipool.tile([H, T], bf16, tag="phi1")
                nc.scalar.activation(phi1[:], psB[:],
                                     mybir.ActivationFunctionType.Tanh,
                                     bias=b1c1[:], scale=1.0)

                # ---- DVE tree: 16 -> 8 -> 4 -> 2 per block (bf16 2x mode)
                NB = T // B  # blocks in tile
                l05s = []
                for ci, phi in enumerate((phi0, phi1)):
                    p3 = phi.rearrange("p (nb w) -> p nb w", w=B)
                    s1 = treepool.tile([H, NB, 8], bf16, tag=f"s1c{ci}")
                    nc.vector.tensor_add(s1[:], p3[:, :, 0:8], p3[:, :, 8:16])
                    s2 = treepool.tile([H, NB, 4], bf16, tag=f"s2c{ci}")
                    nc.vector.tensor_add(s2[:], s1[:, :, 0:4], s1[:, :, 4:8])
                    l05 = l05pool.tile([H, NB, 2], bf16, tag=f"l05c{ci}")
                    nc.vector.tensor_add(l05[:], s2[:, :, 0:2], s2[:, :, 2:4])
                    l05s.append(l05)

                # ---- fc2 on half-block sums -> zT [128(l05col), 128(h)]
                zps = zps_pool.tile([SEGS_PER_WIN, H], f32)
                nc.tensor.matmul(zps[:], lhsT=l05s[0].rearrange("p a b -> p (a b)"),
                                 rhs=w2t0[:], start=True, stop=False)
                nc.tensor.matmul(zps[:], lhsT=l05s[1].rearrange("p a b -> p (a b)"),
                                 rhs=w2t1[:], start=False, stop=True)
                zsb = zpool.tile([SEGS_PER_WIN, H], bf16)
                nc.vector.tensor_copy(zsb[:], zps[:])

                # ---- combine into per-window segment sums (PSUM accumulate)
                st = spool.tile([SEGS_PER_WIN, SEGS_PER_WIN], bf16)
                nc.sync.dma_start(out=st[:], in_=s_d[t])
                if t % lt_per_win == 0:
                    yps = yps_pool.tile([SEGS_PER_WIN, H], f32)
                nc.tensor.matmul(yps[:], lhsT=st[:], rhs=zsb[:],
                                 start=(t % lt_per_win == 0),
                                 stop=(t % lt_per_win == lt_per_win - 1))
                if t % lt_per_win == lt_per_win - 1:
                    ysb = ypool.tile([SEGS_PER_WIN, H], f32)
                    nc.vector.tensor_copy(ysb[:], yps[:])
                    nc.sync.dma_start(
                        out=y_d[w * SEGS_PER_WIN:(w + 1) * SEGS_PER_WIN, :],
                        in_=ysb[:])

    nc.compile()
    return nc


class _Runner:
    """Persistent jitted SPMD executor over jax.devices()[:8]."""

    def __init__(self, nc):
        import jax
        from jax.sharding import Mesh, PartitionSpec
        from jax.experimental.shard_map import shard_map
        from concourse import mybir
        from concourse.bass2jax import (_bass_exec_p, install_neuronx_cc_hook,
                                        partition_id_tensor)
        install_neuronx_cc_hook()
        self.jax = jax
        self.nc = nc
        in_names, out_names, out_avals, zero_outs = [], [], [], []
        partition_name = (nc.partition_id_tensor.name
                          if nc.partition_id_tensor else None)
        for alloc in nc.m.functions[0].allocations:
            if not isinstance(alloc, mybir.MemoryLocationSet):
                continue
            name = alloc.memorylocations[0].name
            if alloc.kind == "ExternalInput":
                if name != partition_name:
                    in_names.append(name)
            elif alloc.kind == "ExternalOutput":
                shape = tuple(alloc.tensor_shape)
                dtype = mybir.dt.np(alloc.dtype)
                out_names.append(name)
                out_avals.append(jax.core.ShapedArray(shape, dtype))
                zero_outs.append(np.zeros(shape, dtype))
        self.in_names, self.out_names = in_names, out_names
        self.out_avals, self.zero_outs = out_avals, zero_outs
        all_in = in_names + out_names + ([partition_name] if partition_name else [])

        def _body(*args):
            operands = list(args)
            if partition_name is not None:
                operands.append(partition_id_tensor())
            return tuple(_bass_exec_p.bind(
                *operands,
                out_avals=tuple(out_avals),
                in_names=tuple(all_in),
                out_names=tuple(out_names),
                lowering_input_output_aliases=(),
                sim_require_finite=True,
                sim_require_nnan=True,
                nc=nc,
            ))

        devices = jax.devices()[:N_CORES]
        self.mesh = Mesh(np.asarray(devices), ("core",))
        n_args = len(in_names) + len(out_names)
        self.fn = jax.jit(
            shard_map(_body, mesh=self.mesh,
                      in_specs=(PartitionSpec("core"),) * n_args,
                      out_specs=(PartitionSpec("core"),) * len(out_names),
                      check_rep=False),
            keep_unused=True,
        )

    def place_inputs(self, in_maps):
        from jax.sharding import NamedSharding, PartitionSpec
        sharding = NamedSharding(self.mesh, PartitionSpec("core"))
        args = []
        for name in self.in_names:
            concat = np.concatenate(
                [np.asarray(m[name]) for m in in_maps], axis=0)
            args.append(self.jax.device_put(concat, sharding))
        for z in self.zero_outs:
            concat = np.zeros((N_CORES * z.shape[0], *z.shape[1:]), z.dtype)
            args.append(self.jax.device_put(concat, sharding))
        return args

    def run(self, args):
        outs = self.fn(*args)
        self.jax.block_until_ready(outs)
        return outs

    def results(self, outs):
        res = []
        for c in range(N_CORES):
            d = {}
            for i, name in enumerate(self.out_names):
                d[name] = np.asarray(outs[i]).reshape(
                    N_CORES, *self.out_avals[i].shape)[c]
            res.append(d)
        return res


_CACHE = {}


def _get_runner(Nc, ntiles):
    key = (Nc, ntiles)
    if key not in _CACHE:
        nc = _build_program(Nc, ntiles)
        _CACHE[key] = _Runner(nc)
    return _CACHE[key]


def _make_in_maps(x, batch, W1, b1, W2):
    xT, S, Nc, ntiles, npad = _prep_host(x, batch)
    W1 = np.asarray(W1, np.float32)
    W2 = np.asarray(W2, np.float32)
    b1 = np.asarray(b1, np.float32)
    w1t = np.ascontiguousarray(W1.T).astype(_BF16)       # [128, 256]
    w2t = np.ascontiguousarray(W2.T).reshape(2, H, H).astype(_BF16)
    b1c = b1.reshape(2, H, 1)
    in_maps = []
    for c in range(N_CORES):
        in_maps.append({
            "xt": xT[c], "w1t": w1t, "w2t": w2t, "b1c": b1c, "smat": S[c],
        })
    return in_maps, Nc, ntiles, npad


def kernel(x, batch, W1, b1, W2, b2):
    x = np.asarray(x, np.float32)
    batch_np = np.asarray(batch)
    b1_np = np.asarray(b1, np.float32)
    b2_np = np.asarray(b2, np.float32)
    W2_np = np.asarray(W2, np.float32)

    in_maps, Nc, ntiles, npad = _make_in_maps(x, batch_np, W1, b1_np, W2_np)
    runner = _get_runner(Nc, ntiles)
    args = runner.place_inputs(in_maps)
    outs = runner.run(args)
    res = runner.results(outs)

    y = np.concatenate([res[c]["y"] for c in range(N_CORES)], axis=0)  # [4096,128]
    corr = np.tanh(b1_np.astype(np.float64)) @ W2_np.astype(np.float64).T  # [128]
    y = y + b2_np[None, :] - npad[:, None] * corr[None, :].astype(np.float32)
    return y.astype(np.float32)
